# revision 17
# baseline (speedup 1.0000x reference)
"""DigitCaps dynamic-routing kernel for 8x Trainium2 NeuronCores.

Full inputs -> batch-sharded across 8 cores (16 samples/core), W replicated.

Per-core layout (partition p = r_l*8 + b, with r_l in [0,16), b in [0,8);
batch half h in {0,1} handles sample 8h+b):

  u_hat[p=128, h=2, g=128, o=16, c=10]  bf16 in SBUF (g = 16-route group)
  b_ij / c_ij: (128p, 2h, 128g, 10c)

u_hat build: per (g, h) one K=128 PE matmul:
  lhsT = xblk[g,h]  (K=128=(r_l,i), M=128=(r_l',b))  block-diagonal x
  rhs  = Wt[g]      (K=128, N=160=(o,c))             shared across h
  out  = psum (128, 160) -> ACT-copy to SBUF bf16 (3 groups per bank)

s0 = 0.1*sum_r u_hat computed DIRECTLY from x,W (independent of u_hat):
  128 accumulating matmuls lhsT=xk[m] (128,16=b), rhs=Wt[m] -> (16,160),
  so v0 finishes as soon as the W DMA lands and routing iteration 1
  overlaps the u_hat build.  v0 broadcast to all partitions via mask MMs.

agreement a = sum_o u_hat*v: DVE computes pa = u_hat*v (bf16), written in
(o, g, c) layout; the o-reduction runs on the PE as 16 accumulating
identity-lhsT matmuls into PSUM (f32 accumulate, frees the DVE).

s_j = sum_r c*u_hat: PE matmuls with lhsT = bmask (delta on b=p%8), summing
over r_l partitions while replicating the result; per h in its own bank.

softmax over c: free-dim innermost; exp on ACT; squash: tiny ops.
"""

import sys

for p in ("/opt/trn_rl_repo",):
    if p not in sys.path:
        sys.path.insert(0, p)

import numpy as np
import ml_dtypes

import concourse.bass as bass
import concourse.bacc as bacc
import concourse.mybir as mybir
import concourse.tile as tile
from concourse.bass_utils import run_bass_kernel_spmd

# Problem constants (hardcoded per contract)
B_FULL = 128
N_CORES = 8
B = B_FULL // N_CORES   # 16 samples per core
R = 2048
C = 10
O = 16
I = 8
ITERS = 3

RG = 16                 # routes per matmul group (K = RG*I = 128)
G = R // RG             # 128 groups
K = RG * I              # 128 contraction rows
CO = C * O              # 160
CH = 32                 # groups per routing chunk
NCH = G // CH           # 4 chunks
NDC = 16                # DMA chunks for xblk (8 groups each)
DCG = G // NDC          # 8 groups per DMA chunk
GCO = CH * C            # 320 = agreement row width per chunk

F32 = mybir.dt.float32
BF16 = mybir.dt.bfloat16

_COMPILED = None


def _host_prep(x, W):
    """Build per-core DMA-ready arrays. x: (128,2048,8) W: (2048,10,16,8)."""
    x = np.ascontiguousarray(x, dtype=np.float32)
    W = np.ascontiguousarray(W, dtype=np.float32)

    # Wt[k=(r_l,i), g, (o,c)] = W[16g+r_l, c, o, i]
    Wt = (W.reshape(G, RG, C, O, I).transpose(1, 4, 0, 3, 2)
          .reshape(K, G, CO))
    Wt = np.ascontiguousarray(Wt).astype(ml_dtypes.bfloat16)

    # bmask[p, p'] = 1 if p%8 == p'%8   (sums r_l, replicates to all parts)
    bmask = np.tile(np.eye(8, dtype=np.float32), (16, 16)).astype(
        ml_dtypes.bfloat16)
    ident = np.eye(128, dtype=np.float32).astype(ml_dtypes.bfloat16)

    # smask[b_full, h, p'] = 1 if b_full == 8h + p'%8  (v0 broadcast)
    smask = np.zeros((B, 2, 128), dtype=np.float32)
    for bf in range(B):
        h, b = divmod(bf, 8)
        smask[bf, h, :] = (np.arange(128) % 8 == b).astype(np.float32)
    smask = smask.astype(ml_dtypes.bfloat16)

    # s0sel[p=16j+b, j, b] = 1  (extracts+sums the two diagonal blocks
    # of the pair-packed s0 accumulator)
    s0sel = np.zeros((32, 2, B), dtype=np.float32)
    for j in range(2):
        for b in range(B):
            s0sel[16 * j + b, j, b] = 1.0
    s0sel = s0sel.astype(ml_dtypes.bfloat16)

    # one packed consts tensor -> a single DMA whose only consumer is one
    # copy, so its HWDGE semaphore lane recycles immediately
    consts = np.zeros((128, 544), dtype=ml_dtypes.bfloat16)
    consts[:, 0:128] = bmask
    consts[:, 128:256] = ident
    consts[0:B, 256:512] = smask.reshape(B, 256)
    consts[0:32, 512:544] = s0sel.reshape(32, 32)

    ar = np.arange(RG)
    shards = []
    for ci in range(N_CORES):
        xs = x[ci * B:(ci + 1) * B]                # (16, 2048, 8)
        # xblk[k=(r_l,i), g, h, col=(r_l',b)] = xs[8h+b, 16g+r_l, i] * delta
        xs2 = xs.reshape(2, 8, G, RG, I)           # (h, b, g, r_l, i)
        Bm = xs2.transpose(3, 4, 2, 0, 1)          # (r_l, i, g, h, b)
        A = np.zeros((RG, I, G, 2, RG, 8), dtype=np.float32)
        A[ar, :, :, :, ar, :] = Bm
        xb = A.reshape(K, G, 2, 128).astype(ml_dtypes.bfloat16)
        # xk[k=(r_l,i), m, b_full] = xs[b_full, 16m+r_l, i]
        xk = (xs.reshape(B, G, RG, I).transpose(2, 3, 1, 0)
              .reshape(K, G, B).astype(ml_dtypes.bfloat16))
        shards.append((np.ascontiguousarray(xb), np.ascontiguousarray(xk)))
    return shards, Wt, consts


def _squash(nc, work, s_sb, sq, sq2, v16, eps_t, nh):
    """v = s * (|s|^2/(1+|s|^2)) / sqrt(|s|^2 + 1e-8), per (h, c).

    s_sb layout (P, nh, O, C) f32; leaves the scale factor in `sq`
    (P, nh, C); v16 = s * scale (bf16)."""
    P = s_sb.shape[0]
    ssq = work.tile([P, nh, O, C], F32, tag="ssq")
    nc.vector.tensor_mul(ssq[:], s_sb[:], s_sb[:])
    nc.vector.reduce_sum(sq[:], ssq[:].rearrange("p h o c -> p h c o"),
                         axis=mybir.AxisListType.X)
    nc.scalar.activation(sq2[:], sq[:], mybir.ActivationFunctionType.Sqrt,
                         bias=eps_t[0:P])
    nc.vector.scalar_tensor_tensor(
        sq2[:], sq[:], 1.0, sq2[:],
        op0=mybir.AluOpType.add, op1=mybir.AluOpType.mult)
    nc.vector.reciprocal(sq2[:], sq2[:])
    nc.vector.tensor_mul(sq[:], sq[:], sq2[:])
    nc.vector.tensor_mul(
        v16[:], s_sb[:],
        sq[:].unsqueeze(2).broadcast_to((P, nh, O, C)))


def _build_kernel():
    nc = bacc.Bacc("TRN2", target_bir_lowering=False, debug=False,
                   num_devices=N_CORES)

    xb_d = nc.dram_tensor("xb", [K, G, 2, 128], BF16, kind="ExternalInput")
    xk_d = nc.dram_tensor("xk", [K, G, B], BF16, kind="ExternalInput")
    wt_d = nc.dram_tensor("wt", [K, G, CO], BF16, kind="ExternalInput")
    cs_d = nc.dram_tensor("consts", [128, 544], BF16, kind="ExternalInput")
    vout_d = nc.dram_tensor("vout", [B, O, C], F32, kind="ExternalOutput")

    Exp = mybir.ActivationFunctionType.Exp

    with tile.TileContext(nc) as tc:
        with (
            tc.tile_pool(name="persist", bufs=1) as persist,
            tc.tile_pool(name="xbl", bufs=4) as xbl,
            tc.tile_pool(name="big", bufs=4) as big,
            tc.tile_pool(name="work", bufs=2) as work,
            tc.tile_pool(name="psum", bufs=1, space="PSUM") as psp,
        ):
            wt_sb = persist.tile([K, G, CO], BF16)        # 40 KiB/part
            xk_sb = persist.tile([K, G, B], BF16)         # 4 KiB
            uhat = persist.tile([128, 2, G, O, C], BF16)  # 80 KiB
            bij = persist.tile([128, 2, G, C], BF16)      # 5 KiB
            cst_raw = persist.tile([128, 544], BF16)
            cst = persist.tile([128, 544], BF16)
            p32sb = persist.tile([32, 2, CO], BF16)
            v_bf = persist.tile([128, 2, O, C], BF16)
            s_sb = persist.tile([128, 2, O, C], F32)
            sq = persist.tile([128, 2, C], F32)
            sq2 = persist.tile([128, 2, C], F32)
            s0f = persist.tile([B, 1, O, C], F32)
            v0bf = persist.tile([B, 1, O, C], BF16)
            sq0 = persist.tile([B, 1, C], F32)
            sq02 = persist.tile([B, 1, C], F32)
            eps_t = persist.tile([128, 1], F32)
            nc.gpsimd.memset(eps_t[:], 1e-8)

            # ---------- DMA: consts (one transfer, freed by one copy),
            # xk, then W in exactly two big halves (one per HWDGE queue).
            # Keeping the DMA count low matters: Tile has 8 HWDGE sem
            # lanes and a lane only recycles after the previous DMA's
            # consumers complete.
            nc.sync.dma_start(cst_raw[:], cs_d[:])
            nc.vector.tensor_copy(cst[:], cst_raw[:])
            bmask_b = cst[:, 0:128]
            ident_b = cst[:, 128:256]
            nc.scalar.dma_start(xk_sb[:], xk_d[:])
            nc.sync.dma_start(wt_sb[:, 0:32, :], wt_d[:, 0:32, :])
            nc.scalar.dma_start(wt_sb[:, 32:64, :], wt_d[:, 32:64, :])

            # ---------- u_hat build chunks ------------------------------
            def build_chunk(dc):
                xbt = xbl.tile([K, DCG, 2, 128], BF16, tag="xbt",
                               name=f"xbt{dc}")
                eng = nc.sync if dc % 2 == 0 else nc.scalar
                eng.dma_start(xbt[:], xb_d[:, DCG * dc:DCG * (dc + 1), :, :])
                g0 = DCG * dc
                for h in range(2):
                    for t0 in range(0, DCG, 3):
                        n = min(3, DCG - t0)
                        pbt = psp.tile([128, 3, CO], F32, tag="pb", bufs=2,
                                       name=f"pb{dc}_{h}_{t0}")
                        for j in range(n):
                            nc.tensor.matmul(
                                pbt[:, j, :],
                                lhsT=xbt[:, t0 + j, h, :],
                                rhs=wt_sb[:, g0 + t0 + j, :],
                                start=True, stop=True,
                                skip_group_check=True)
                        nc.scalar.copy(
                            uhat[:, h, g0 + t0:g0 + t0 + n, :, :]
                            .rearrange("p g o c -> p (g o c)"),
                            pbt[:, 0:n, :].rearrange("p g n -> p (g n)"))

            # ---------- PE warm-up: wake the HAM clock gate before s0 ---
            warm = psp.tile([128, GCO], F32, tag="ap", bufs=3, name="warm")
            for i in range(90):
                nc.tensor.matmul(warm[:, 0:128], lhsT=ident_b,
                                 rhs=ident_b, start=True, stop=True,
                                 skip_group_check=True)

            # ---------- s0 = 0.1 * sum_r u_hat, direct from x, W -------
            # pair-packed: lhsT covers two 16-route blocks (M=32), rhs two
            # W blocks (N=320); the two diagonal (b x (o,c)) blocks are the
            # real partial sums, extracted and summed by two sel-matmuls.
            # Builds for the first iteration chunk are interleaved so the
            # PE starts them as soon as their xblk DMA lands.
            p32 = psp.tile([32, 2, CO], F32, tag="s0", bufs=1)

            def s0_part(part):
                for m in range(16 * part, 16 * part + 16):
                    nc.tensor.matmul(
                        p32[:], lhsT=xk_sb[:, 2 * m:2 * m + 2, :].rearrange(
                            "p g b -> p (g b)"),
                        rhs=wt_sb[:, 2 * m:2 * m + 2, :].rearrange(
                            "p g n -> p (g n)"),
                        start=(m == 0), stop=(m == G // 2 - 1),
                        skip_group_check=True)

            nc.sync.dma_start(wt_sb[:, 64:96, :], wt_d[:, 64:96, :])
            nc.scalar.dma_start(wt_sb[:, 96:G, :], wt_d[:, 96:G, :])
            s0_part(0)
            build_chunk(0)
            build_chunk(1)
            s0_part(1)
            build_chunk(2)
            build_chunk(3)
            s0_part(2)
            s0_part(3)
            nc.scalar.copy(p32sb[:], p32[:])
            s0ps = psp.tile([B, O, C], F32, tag="sp", bufs=1, name="s0ps")
            for j in range(2):
                nc.tensor.matmul(
                    s0ps[:], lhsT=cst[0:32, 512 + B * j:512 + B * (j + 1)],
                    rhs=p32sb[:, j, :],
                    start=(j == 0), stop=(j == 1),
                    skip_group_check=True)
            nc.scalar.activation(s0f[:], s0ps[:],
                                 mybir.ActivationFunctionType.Copy,
                                 scale=0.1)
            _squash(nc, work, s0f, sq0, sq02, v0bf, eps_t, 1)
            # broadcast v0 to all 128 partitions: psum <- smask_h.T @ v0
            for h in range(2):
                v0ps = psp.tile([128, GCO], F32, tag="ap", bufs=3,
                                name=f"v0ps{h}")
                nc.tensor.matmul(
                    v0ps[:, 0:CO], lhsT=cst[0:B, 256 + 128 * h:256 + 128 * (h + 1)],
                    rhs=v0bf[:].rearrange("p h o c -> p (h o c)"),
                    start=True, stop=True, skip_group_check=True)
                nc.vector.tensor_copy(
                    v_bf[:, h], v0ps[:, 0:CO].rearrange(
                        "p (o c) -> p o c", o=O))

            # ---------- routing iteration stages ------------------------
            def stage_A_tree(it, ch):
                """pa = u_hat*v (DVE); o-reduce fully on DVE (tree);
                writes the agreement straight into bij (it1) ."""
                sl = slice(ch * CH, ch * CH + CH)
                for h in range(2):
                    pa = big.tile([128, O, CH, C], BF16, tag="big",
                                  name=f"pa{it}_{ch}_{h}")
                    nc.vector.tensor_mul(
                        pa[:].rearrange("p o g c -> p g o c"),
                        uhat[:, h, sl],
                        v_bf[:, h].unsqueeze(1)
                        .broadcast_to((128, CH, O, C)))
                    nc.vector.tensor_add(pa[:, 0:8], pa[:, 0:8], pa[:, 8:16])
                    nc.vector.tensor_add(pa[:, 0:4], pa[:, 0:4], pa[:, 4:8])
                    nc.vector.tensor_add(pa[:, 0:2], pa[:, 0:2], pa[:, 2:4])
                    nc.vector.tensor_add(
                        bij[:, h, sl],
                        pa[:, 0].rearrange("p g c -> p g c"),
                        pa[:, 1].rearrange("p g c -> p g c"))

            def stage_A_pe(it, ch):
                """pa = u_hat*v (DVE); one DVE tree level (16->8 o), then
                8 accumulating identity matmuls on PE -> apsum (f32)."""
                sl = slice(ch * CH, ch * CH + CH)
                aps = []
                for h in range(2):
                    pa = big.tile([128, O, CH, C], BF16, tag="big",
                                  name=f"pa{it}_{ch}_{h}")
                    nc.vector.tensor_mul(
                        pa[:].rearrange("p o g c -> p g o c"),
                        uhat[:, h, sl],
                        v_bf[:, h].unsqueeze(1)
                        .broadcast_to((128, CH, O, C)))
                    nc.vector.tensor_add(pa[:, 0:8], pa[:, 0:8], pa[:, 8:16])
                    ap = psp.tile([128, GCO], F32, tag="ap", bufs=3,
                                  name=f"ap{it}_{ch}_{h}")
                    for o in range(8):
                        nc.tensor.matmul(
                            ap[:], lhsT=ident_b,
                            rhs=pa[:, o].rearrange("p g c -> p (g c)"),
                            start=(o == 0), stop=(o == 7),
                            skip_group_check=True)
                    aps.append(ap)
                return aps

            def stage_bij(it, ch, aps):
                """bij += a (from PE-agree psum)."""
                sl = slice(ch * CH, ch * CH + CH)
                for h in range(2):
                    av = aps[h][:].rearrange("p (g c) -> p g c", g=CH)
                    nc.vector.tensor_add(bij[:, h, sl], bij[:, h, sl], av)

            def stage_CDE(it, ch, spt, cexp, cbf, zsum):
                """softmax tail (DVE) + prods (DVE) + s-sum MMs (PE)."""
                sl = slice(ch * CH, ch * CH + CH)
                nc.scalar.activation(cexp[:], bij[:, :, sl], Exp)
                nc.vector.reduce_sum(zsum[:], cexp[:],
                                     axis=mybir.AxisListType.X)
                nc.vector.reciprocal(zsum[:], zsum[:])
                nc.vector.tensor_mul(
                    cbf[:], cexp[:],
                    zsum[:].unsqueeze(3).broadcast_to((128, 2, CH, C)))
                for h in range(2):
                    prods = big.tile([128, CH, O, C], BF16, tag="big",
                                     name=f"px{it}_{ch}_{h}")
                    nc.vector.tensor_mul(
                        prods[:], uhat[:, h, sl],
                        cbf[:, h].unsqueeze(2)
                        .broadcast_to((128, CH, O, C)))
                    # 10 triples + one pair per 32-group chunk
                    for t in range(10):
                        nc.tensor.matmul(
                            spt[:, h, 0:3 * CO], lhsT=bmask_b,
                            rhs=prods[:, 3 * t:3 * t + 3].rearrange(
                                "p g o c -> p (g o c)"),
                            start=(ch == 0 and t == 0), stop=False,
                            skip_group_check=True)
                    nc.tensor.matmul(
                        spt[:, h, 0:2 * CO], lhsT=bmask_b,
                        rhs=prods[:, 30:32].rearrange("p g o c -> p (g o c)"),
                        start=False, stop=(ch == NCH - 1),
                        skip_group_check=True)

            def s_combine(spt):
                nc.scalar.copy(
                    s_sb[:],
                    spt[:, :, 0:CO].rearrange("p h (o c) -> p h o c", o=O))
                for t in range(1, 3):
                    nc.vector.tensor_add(
                        s_sb[:], s_sb[:],
                        spt[:, :, t * CO:(t + 1) * CO].rearrange(
                            "p h (o c) -> p h o c", o=O))

            def exp_tiles(it, ch):
                cexp = work.tile([128, 2, CH, C], BF16, tag="cexp",
                                 name=f"ce{it}_{ch}")
                cbf = work.tile([128, 2, CH, C], BF16, tag="cbf",
                                name=f"cb{it}_{ch}")
                zsum = work.tile([128, 2, CH], F32, tag="zs",
                                 name=f"zs{it}_{ch}")
                return cexp, cbf, zsum

            # ---------- iteration 1 (fused with the u_hat build) -------
            # agreement o-reduce fully on DVE (the PE is busy building)
            spt = psp.tile([128, 2, 512], F32, tag="sp", bufs=1, name="sp1")
            prev = None
            for ch in range(NCH):
                for dc in range(4 + 3 * ch, 7 + 3 * ch):
                    build_chunk(dc)
                stage_A_tree(1, ch)
                if prev is not None:
                    stage_CDE(1, prev[0], spt, *prev[1])
                prev = (ch, exp_tiles(1, ch))
            stage_CDE(1, prev[0], spt, *prev[1])
            s_combine(spt)
            _squash(nc, work, s_sb, sq, sq2, v_bf, eps_t, 2)

            # ---------- iteration 2 -------------------------------------
            # agreement split: one DVE tree level + 8 PE chains per half
            spt = psp.tile([128, 2, 512], F32, tag="sp", bufs=1, name="sp2")
            prev = None
            for ch in range(NCH):
                aps = stage_A_pe(2, ch)
                if prev is not None:
                    stage_CDE(2, prev[0], spt, *prev[1])
                stage_bij(2, ch, aps)
                prev = (ch, exp_tiles(2, ch))
            stage_CDE(2, prev[0], spt, *prev[1])
            s_combine(spt)
            _squash(nc, work, s_sb, sq, sq2, v_bf, eps_t, 2)

            # ---------- output ----------
            vfin = work.tile([128, 2, O, C], F32, tag="vfin")
            nc.vector.tensor_mul(
                vfin[:], s_sb[:],
                sq[:].unsqueeze(2).broadcast_to((128, 2, O, C)))
            for h in range(2):
                nc.sync.dma_start(vout_d[8 * h:8 * h + 8], vfin[0:8, h])

    nc.compile()
    return nc


def _make_in_maps(x, W):
    shards, Wt, consts = _host_prep(x, W)
    return [
        {"xb": xb, "xk": xk, "wt": Wt, "consts": consts}
        for (xb, xk) in shards
    ]


def kernel(x, W):
    global _COMPILED
    if _COMPILED is None:
        _COMPILED = _build_kernel()
    nc = _COMPILED
    in_maps = _make_in_maps(x, W)
    res = run_bass_kernel_spmd(nc, in_maps, list(range(N_CORES)))
    outs = []
    for ci in range(N_CORES):
        v = res.results[ci]["vout"]  # (16, O, C)
        outs.append(v.transpose(0, 2, 1))  # -> (16, C, O)
    return np.ascontiguousarray(np.concatenate(outs, axis=0), dtype=np.float32)
